# revision 9
# baseline (speedup 1.0000x reference)
"""Trainium2 Bass kernel for nn_Antecedents: fuzzy-rule antecedent activations.

Computes out[n, r] = prod_v memberships[v, n, set_v(r)] over the full
Cartesian product of fuzzy sets (R = 4**6 = 4096 rules), for N = 16384
samples, data-parallel over 8 NeuronCores (2048 samples per core).

Per-core layout: sample n = p*16 + j (p = SBUF partition 0..127,
j = 0..15).  The rule index splits little-endian-last as
r = s0*1024 + s1*256 + s2*64 + s3*16 + s4*4 + s5.

Bottleneck model (measured): the per-core DMA subsystem sustains
~410 GB/s (16 engines) regardless of packet size or queue count, so
shipping the 16 MB bf16 output shard takes >= 41 us.  The kernel keeps
production above that rate with the *repeated-scalar TT* trick: a
[128, 4096] TENSOR_TENSOR computing a512[c] * x01[s0s1] where the
per-(s0,s1) scalars live in a tiny bf16 tile with each value stored
TWICE, so every operand's innermost AP dim is packed 2-byte pairs ->
the op runs in the DVE 2x perf mode (2.29 us per j-block of 1 MB,
one instruction).  Five j-blocks run on ACT instead (activation-Copy
with per-partition scale from a1024[j], itself one repeated-scalar TT)
to keep DVE under the stream time.  GpSimd is used only for tiny
memsets: its TT/TS ops starve concurrent DVE 4x/2x ops (measured 4x
slowdown via SBUF contention).

Output is stored bf16 (<= 4 bf16 roundings, max rel err ~1.2e-2 vs the
2e-2 gate), halving output-write traffic; the host gather casts back
to float32.
"""

import numpy as np
from contextlib import ExitStack

import concourse.bass as bass
import concourse.tile as tile
from concourse import bacc, mybir
from concourse.bass_utils import run_bass_kernel_spmd

N_VARS = 6
N_FULL = 16384
N_SETS = 4
N_CORES = 8
N_SHARD = N_FULL // N_CORES  # 2048
P = 128
J = N_SHARD // P             # 16 samples per partition
R = N_SETS ** N_VARS         # 4096
F32 = mybir.dt.float32
BF16 = mybir.dt.bfloat16
MUL = mybir.AluOpType.mult

LAST_RESULTS = None
_CACHE = {}

ACT_JS = (3, 6, 9, 12, 15)  # j-blocks produced by the ACT engine


def _bap(tile_ap, col_off, dims):
    """AP into a [P, W] tile starting at column col_off with explicit
    free dims [(stride_elems, count), ...] (outer -> inner; stride 0 =
    broadcast)."""
    base = tile_ap[:]
    return bass.AP(
        tensor=base.tensor,
        offset=base.offset + col_off,
        ap=[base.ap[0]] + [[s, c] for (s, c) in dims],
    )


def build_nc():
    nc = bacc.Bacc(
        "TRN2", target_bir_lowering=False, debug=False, num_devices=N_CORES
    )
    m = nc.dram_tensor(
        "memberships", [N_VARS, N_SHARD, N_SETS], F32, kind="ExternalInput"
    ).ap()
    out = nc.dram_tensor("out", [N_SHARD, R], BF16, kind="ExternalOutput").ap()
    out_v = out.rearrange("(p f) r -> p (f r)", p=P)  # [128, J*R]

    with tile.TileContext(nc) as tc, ExitStack() as ctx:
        pool = ctx.enter_context(tc.tile_pool(name="all", bufs=1))

        # ACT activation-table preload off the critical path.
        warm = pool.tile([P, 1], F32, tag="warm")
        nc.gpsimd.memset(warm[:], 0.0)
        nc.scalar.activation(
            warm[:], warm[:], mybir.ActivationFunctionType.Copy
        )
        ones = pool.tile([P, 1], F32, tag="ones")
        nc.gpsimd.memset(ones[:], 1.0)

        # Input: three dual-variable DMAs, (v4,v5) first (feeds the
        # first TT), then (v2,v3), then (v0,v1).
        # X[v]: [128, 64] f32, column j*4 + s  <-  memberships[v, p*16+j, s]
        xva = pool.tile([P, 2 * J * N_SETS], F32, tag="xva")
        xvb = pool.tile([P, 2 * J * N_SETS], F32, tag="xvb")
        xvc = pool.tile([P, 2 * J * N_SETS], F32, tag="xvc")

        def m_vars(v0):
            return bass.AP(
                tensor=m.tensor,
                offset=m.offset + v0 * N_SHARD * N_SETS,
                ap=[
                    [J * N_SETS, P],
                    [N_SHARD * N_SETS, 2],
                    [N_SETS, J],
                    [1, N_SETS],
                ],
            )

        nc.sync.dma_start(out=xva[:], in_=m_vars(4))
        nc.sync.dma_start(out=xvb[:], in_=m_vars(2))
        nc.sync.dma_start(out=xvc[:], in_=m_vars(0))
        X = {4: (xva, 0), 5: (xva, 64), 2: (xvb, 0), 3: (xvb, 64),
             0: (xvc, 0), 1: (xvc, 64)}

        def xcol(v, j, s):
            t, base = X[v]
            c = base + j * N_SETS + s
            return t[:, c : c + 1]

        def xap(v, off, dims):
            t, base = X[v]
            return _bap(t, base + off, dims)

        # a16_all[:, j*16 + s4*4 + s5] = X4[:, j*4+s4] * X5[:, j*4+s5]
        a16_all = pool.tile([P, J * 16], F32, tag="a16a")
        nc.vector.tensor_tensor(
            out=a16_all[:].rearrange("p (j a b) -> p j a b", j=J, a=4),
            in0=xap(4, 0, [(4, J), (1, 4), (0, 4)]),
            in1=xap(5, 0, [(4, J), (0, 4), (1, 4)]),
            op=MUL,
        )
        # x23[:, j*16 + s2*4 + s3] = X2[:, j*4+s2] * X3[:, j*4+s3]
        x23 = pool.tile([P, J * 16], F32, tag="x23")
        nc.vector.tensor_tensor(
            out=x23[:].rearrange("p (j a b) -> p j a b", j=J, a=4),
            in0=xap(2, 0, [(4, J), (1, 4), (0, 4)]),
            in1=xap(3, 0, [(4, J), (0, 4), (1, 4)]),
            op=MUL,
        )

        # a512 pairs: a512p[u][:, t'*512 + jj*256 + g*16 + k] =
        #   a16_all[:, (2(2u+t')+jj)*16 + k] * x23[:, (...)*16 + g]
        # (u covers js 4u..4u+3; one [128,1024] f32 TT each)
        a512p = [
            pool.tile([P, 1024], BF16, tag=f"a512p_{u}", name=f"a512p_{u}")
            for u in range(4)
        ]

        def make_a512p(u):
            # (t, jj) merge into one uniform-stride-16 dim of count 4
            nc.vector.tensor_tensor(
                out=a512p[u][:].rearrange(
                    "p (tj g k) -> p tj g k", tj=4, g=16
                ),
                in0=_bap(a16_all, u * 64, [(16, 4), (0, 16), (1, 16)]),
                in1=_bap(x23, u * 64, [(16, 4), (1, 16), (0, 16)]),
                op=MUL,
            )

        def a512_half(j):
            # [tile, column offset of the 256-wide half for this j]
            return a512p[j // 4], (j % 4) * 256

        # x01rep[:, j*32 + (s0*4+s1)*2 + {0,1}] = X0[j,s0] * X1[j,s1]
        # (each scalar stored twice -> packed bf16 pairs for 2x mode).
        # j0's block is a tiny pure-f32 TT on the ramp critical path;
        # j1..15 come from X0 * x1rep (one extra bf16 rounding, fine).
        x01rep = pool.tile([P, J * 32], BF16, tag="x01rep")
        # x1rep[:, j*8 + s1*2 + {0,1}] = X1[j,s1]  (bf16, for a1024 TTs)
        x1rep = pool.tile([P, J * 8], BF16, tag="x1rep")

        def make_x01rep_j(j):
            nc.vector.tensor_tensor(
                out=_bap(x01rep, j * 32, [(8, 4), (2, 4), (1, 2)]),
                in0=xap(0, j * 4, [(1, 4), (0, 4), (0, 2)]),
                in1=xap(1, j * 4, [(0, 4), (1, 4), (0, 2)]),
                op=MUL,
            )

        def make_x1rep():
            nc.vector.tensor_scalar_mul(
                x1rep[:].rearrange("p (j a t) -> p j a t", j=J, a=4),
                xap(1, 0, [(4, J), (1, 4), (0, 2)]),
                ones[:, 0:1],
            )

        def make_x01rep_rest():
            # x01rep[j=1..15] = X0[j,s0] * x1rep[j, (s1,rep)]
            nc.vector.tensor_tensor(
                out=_bap(x01rep, 32, [(32, J - 1), (8, 4), (1, 8)]),
                in0=xap(0, 4, [(4, J - 1), (1, 4), (0, 8)]),
                in1=_bap(x1rep, 8, [(8, J - 1), (0, 4), (1, 8)]),
                op=MUL,
            )

        ot = [
            pool.tile([P, R], BF16, tag=f"ot_{j}", name=f"ot_{j}")
            for j in range(J)
        ]
        a1024 = {
            j: pool.tile([P, 1024], BF16, tag=f"a1024_{j}", name=f"a1024_{j}")
            for j in ACT_JS
        }

        def tt_j(j, half=None):
            # ot[j][:, (s0s1)*256 + c] = a512[j][c] * x01rep[j][s0s1]
            # One DVE 2x-mode TT per (half-)block.
            src, off = a512_half(j)
            if half is None:
                s_cnt, s_off = 16, 0
            else:
                s_cnt, s_off = 8, half * 8
            nc.vector.tensor_tensor(
                out=_bap(ot[j], s_off * 256, [(256, s_cnt), (2, 128), (1, 2)]),
                in0=_bap(src, off, [(0, s_cnt), (2, 128), (1, 2)]),
                in1=_bap(x01rep, j * 32 + s_off * 2, [(2, s_cnt), (0, 128), (1, 2)]),
                op=MUL,
            )

        def make_a1024(j):
            # a1024[j][:, s1*256 + c] = a512[j][c] * x1rep[j][s1]
            src, off = a512_half(j)
            nc.vector.tensor_tensor(
                out=_bap(a1024[j], 0, [(256, 4), (2, 128), (1, 2)]),
                in0=_bap(src, off, [(0, 4), (2, 128), (1, 2)]),
                in1=_bap(x1rep, j * 8, [(2, 4), (0, 128), (1, 2)]),
                op=MUL,
            )

        def act_units(j):
            for s0 in range(N_SETS):
                nc.scalar.activation(
                    ot[j][:, 1024 * s0 : 1024 * (s0 + 1)],
                    a1024[j][:],
                    mybir.ActivationFunctionType.Copy,
                    scale=xcol(0, j, s0),
                )

        def ship(j, n_chunks=1):
            w = R // n_chunks
            for c in range(n_chunks):
                nc.sync.dma_start(
                    out=out_v[:, j * R + c * w : j * R + (c + 1) * w],
                    in_=ot[j][:, c * w : (c + 1) * w],
                )

        # --- DVE emission order (ramp-first, then steady cadence) ---
        make_a512p(0)          # js 0..3
        make_x01rep_j(0)
        tt_j(0, half=0)
        ship_part = lambda j, c: nc.sync.dma_start(
            out=out_v[:, j * R + c * 2048 : j * R + (c + 1) * 2048],
            in_=ot[j][:, c * 2048 : (c + 1) * 2048],
        )
        ship_part(0, 0)
        tt_j(0, half=1)
        ship_part(0, 1)
        make_x1rep()
        make_x01rep_rest()
        tt_j(1)
        ship(1)
        tt_j(2)
        ship(2)
        make_a1024(3)
        act_units(3)
        make_a512p(1)          # js 4..7
        tt_j(4)
        ship(4)
        ship(3)
        tt_j(5)
        ship(5)
        make_a1024(6)
        act_units(6)
        tt_j(7)
        ship(7)
        make_a512p(2)          # js 8..11
        tt_j(8)
        ship(8)
        ship(6)
        make_a1024(9)
        act_units(9)
        tt_j(10)
        ship(10)
        tt_j(11)
        ship(11)
        make_a512p(3)          # js 12..15
        make_a1024(12)
        act_units(12)
        ship(9)
        tt_j(13)
        ship(13)
        tt_j(14)
        ship(14)
        make_a1024(15)
        act_units(15)
        ship(12)
        ship(15)

    nc.compile()
    return nc


def _get_nc():
    if "nc" not in _CACHE:
        _CACHE["nc"] = build_nc()
    return _CACHE["nc"]


def kernel(memberships):
    global LAST_RESULTS
    m = np.ascontiguousarray(np.asarray(memberships, dtype=np.float32))
    assert m.shape == (N_VARS, N_FULL, N_SETS), m.shape
    nc = _get_nc()
    shards = np.split(m, N_CORES, axis=1)
    in_maps = [{"memberships": np.ascontiguousarray(s)} for s in shards]
    res = run_bass_kernel_spmd(nc, in_maps, core_ids=list(range(N_CORES)))
    LAST_RESULTS = res
    return np.concatenate(
        [res.results[i]["out"] for i in range(N_CORES)], axis=0
    ).astype(np.float32)
